# revision 5
# baseline (speedup 1.0000x reference)
"""Trainium2 Bass kernel for nn_AttentionCompiled (dense transformer attention).

B=8, N=1024, C=768, H=12 heads, D=64. Per-head LayerNorm on q/k, softmax
attention, output projection.

Strategy: pure data parallelism — one batch element per NeuronCore (B=8 ==
n_cores), weights replicated, no collectives.

Math folding (validated vs reference to ~1e-6):
 - LayerNorm centering is folded into the QKV weights: Wq_h <- (I - J/64) Wq_h
   (and same for k). Row-sums of centered vectors vanish, so
   (q-mu_q).(k-mu_k) == qc.kc with both pre-centered by the weights.
 - 1/sigma_q (and the 1/sqrt(D) attention scale) is folded into q before the
   QK matmul: aq[n] = 1/sqrt(sum_d qc^2 + 64*eps) == (1/8)/sqrt(var+eps).
 - 1/sigma_k is applied as the per-partition `scale` operand of the Exp
   activation on S^T tiles (partition axis = m = key index).
 - Softmax has no max-subtraction: |scores| <= 8 by Cauchy-Schwarz on the
   normalized vectors, so exp is always in-range in fp32.
 - Softmax denominators come free from an appended ones-column on V:
   out^T = [v | 1]^T @ P^T gives rows 0..63 = unnormalized output, row 64 =
   sum_m P^T[m, n].
 - All activations live in transposed ([feature, token]) layouts so the
   attention chain needs zero on-chip transposes; x is pre-transposed on the
   CPU (free), and the final projection (lhsT = O^T slice) lands the output
   back in natural [token, feature] layout.
"""

import sys
import numpy as np
from contextlib import ExitStack

if "/opt/trn_rl_repo" not in sys.path:
    sys.path.insert(0, "/opt/trn_rl_repo")

import concourse.bass as bass
import concourse.bacc as bacc
import concourse.tile as tile
from concourse import mybir

F32 = mybir.dt.float32
F32R = mybir.dt.float32r

N = 1024     # tokens per core
C = 768      # channels
H = 12       # heads
D = 64       # head dim
NT = N // 128   # 8 token tiles
CT = C // 128   # 6 channel tiles
NB = N // 512   # 2 free-dim blocks of 512
EPS = 1e-5

USE_F32R = True  # fp32 matmuls run 4 cyc/row; float32r runs 1 cyc/row at N>=256


def build(use_f32r: bool = USE_F32R, compile_module: bool = True,
          repeat: int = 1) -> bass.Bass:
    nc = bacc.Bacc()

    xT = nc.declare_dram_parameter("xT", [C, N], F32, isOutput=False)         # x[b].T
    wq = nc.declare_dram_parameter("wqcT", [C, C], F32, isOutput=False)       # centered Wq^T  [c, hd]
    wk = nc.declare_dram_parameter("wkcT", [C, C], F32, isOutput=False)       # centered Wk^T
    wv = nc.declare_dram_parameter("wvT", [C, C], F32, isOutput=False)        # Wv^T
    wp = nc.declare_dram_parameter("wpT", [C, C], F32, isOutput=False)        # proj_weight^T
    seg = nc.declare_dram_parameter("seg", [128, CT, H], F32, isOutput=False)  # per-c-tile head-segment-sum lhsT
    ind = nc.declare_dram_parameter("ind", [H, C], F32, isOutput=False)       # head->channel block indicator (bcast lhsT)
    id12 = nc.declare_dram_parameter("id12", [H, H], F32, isOutput=False)     # identity for PE transpose
    out_ext = nc.declare_dram_parameter("out", [N, C], F32, isOutput=True)

    MMD = F32R if use_f32r else F32

    def mm(ap):
        # bitcast for APs out of fp32-typed tensors; no-op if already f32r
        if use_f32r and ap.dtype != F32R:
            return ap.bitcast(F32R)
        return ap

    with tile.TileContext(nc) as tc, ExitStack() as ctx:
        # ---- pools ----
        # persistent single-buffer tensors get their own tags in `persist`
        persist = ctx.enter_context(tc.tile_pool(name="persist", bufs=1))
        # xT tiles and O^T tiles share slots (disjoint lifetimes), same for wv/wp
        xo_pool = ctx.enter_context(tc.tile_pool(name="xo", bufs=6))
        vp_pool = ctx.enter_context(tc.tile_pool(name="vp", bufs=6))
        work = ctx.enter_context(tc.tile_pool(name="work", bufs=2))
        ptp = ctx.enter_context(tc.tile_pool(name="ptp", bufs=3))
        epi = ctx.enter_context(tc.tile_pool(name="epi", bufs=2))
        rows = ctx.enter_context(tc.tile_pool(name="rows", bufs=1))
        dramp = ctx.enter_context(tc.tile_pool(name="dramp", bufs=2, space="DRAM"))
        psA = ctx.enter_context(tc.tile_pool(name="psA", bufs=2, space="PSUM"))
        psB = ctx.enter_context(tc.tile_pool(name="psB", bufs=2, space="PSUM"))

        for _it in range(repeat):
            emit_body(nc, tc, use_f32r, mm,
                      xT, wq, wk, wv, wp, seg, ind, id12, out_ext,
                      persist, xo_pool, vp_pool, work, ptp, epi, rows,
                      dramp, psA, psB)

    if compile_module:
        nc.compile()
    return nc


def emit_body(nc, tc, use_f32r, mm,
              xT, wq, wk, wv, wp, seg, ind, id12, out_ext,
              persist, xo_pool, vp_pool, work, ptp, epi, rows,
              dramp, psA, psB):
    MMD = F32R if use_f32r else F32

    # ---- load inputs to SBUF ----
    xt_sb = []
    for r in range(CT):
        t = xo_pool.tile([128, N], MMD, name=f"xt{r}", tag="xo")
        nc.sync.dma_start(out=t, in_=mm(xT[128 * r:128 * (r + 1), :]))
        xt_sb.append(t)

    def load_w(dram, nm, pool, tag, per_tile_tags=False):
        tiles = []
        for r in range(CT):
            t = pool.tile(
                [128, C], MMD, name=f"{nm}{r}",
                tag=(f"{tag}{r}" if per_tile_tags else tag),
            )
            nc.sync.dma_start(out=t, in_=mm(dram[128 * r:128 * (r + 1), :]))
            tiles.append(t)
        return tiles

    wq_sb = load_w(wq, "wq", persist, "wq", per_tile_tags=True)
    wk_sb = load_w(wk, "wk", persist, "wk", per_tile_tags=True)
    wv_sb = load_w(wv, "wv", vp_pool, "vp")

    seg_sb = rows.tile([128, CT, H], MMD, name="seg", tag="seg")
    nc.sync.dma_start(out=seg_sb, in_=mm(seg[:, :, :]))
    ind_sb = rows.tile([H, C], MMD, name="ind", tag="ind")
    nc.sync.dma_start(out=ind_sb, in_=mm(ind[:, :]))
    id12_sb = rows.tile([H, H], F32, name="id12", tag="id12")
    nc.sync.dma_start(out=id12_sb, in_=id12[:, :])

    epsq_sb = rows.tile([H, 1], F32, name="epsq", tag="epsq")
    nc.vector.memset(epsq_sb, float(D) * EPS)
    epsk_sb = rows.tile([H, 1], F32, name="epsk", tag="epsk")
    nc.vector.memset(epsk_sb, EPS)

    # ---- stage A: q^c.T and k^c.T  ([hd, n] layouts, 6 c-tiles each) ----
    qh_sb, kh_sb = [], []
    for nm, wt, outl in (("qh", wq_sb, qh_sb), ("kh", wk_sb, kh_sb)):
        for r in range(CT):
            ps = psA.tile([128, N], F32, name=f"ps_{nm}{r}", tag="a")
            for kc in range(CT):
                for nb in range(NB):
                    nc.tensor.matmul(
                        ps[:, 512 * nb:512 * (nb + 1)],
                        lhsT=mm(wt[kc][:, 128 * r:128 * (r + 1)]),
                        rhs=mm(xt_sb[kc][:, 512 * nb:512 * (nb + 1)]),
                        start=(kc == 0),
                        stop=(kc == CT - 1),
                    )
            t = persist.tile([128, N], MMD, name=f"{nm}sb{r}", tag=f"{nm}{r}")
            nc.vector.tensor_copy(out=t, in_=ps)
            outl.append(t)

    # ---- stage A2: V in natural layout with ones column: [m, h, 65] ----
    v_sb = []
    for mt in range(NT):
        ps = psA.tile([128, C], F32, name=f"ps_v{mt}", tag="a")
        for kc in range(CT):
            for vo, vn in ((0, 512), (512, 256)):
                nc.tensor.matmul(
                    ps[:, vo:vo + vn],
                    lhsT=mm(xt_sb[kc][:, 128 * mt:128 * (mt + 1)]),
                    rhs=mm(wv_sb[kc][:, vo:vo + vn]),
                    start=(kc == 0),
                    stop=(kc == CT - 1),
                )
        t = persist.tile([128, H, D + 1], MMD, name=f"vsb{mt}", tag=f"v{mt}")
        # ones column for the softmax-denominator trick: fill with 1.0
        # first (contiguous memset), then overwrite cols 0..D-1 with v
        nc.vector.memset(t.bitcast(F32) if use_f32r else t, 1.0)
        nc.vector.tensor_copy(
            out=t[:, :, 0:D], in_=ps.rearrange("p (h d) -> p h d", h=H)
        )
        v_sb.append(t)

    # ---- stage B: per-head inverse std rows  aq, ak [12, 1024] ----
    stat_sb = {}
    for nm, src, eps_t, in (("aq", qh_sb, epsq_sb), ("ak", kh_sb, epsk_sb)):
        ps = psA.tile([H, N], F32, name=f"ps_{nm}", tag="a")
        for r in range(CT):
            sq = work.tile([128, N], MMD, name=f"sq_{nm}{r}", tag="sq")
            nc.vector.tensor_mul(out=sq, in0=src[r], in1=src[r])
            for nb in range(NB):
                nc.tensor.matmul(
                    ps[:, 512 * nb:512 * (nb + 1)],
                    lhsT=mm(seg_sb[:, r, :]),
                    rhs=mm(sq[:, 512 * nb:512 * (nb + 1)]),
                    start=(r == 0),
                    stop=(r == CT - 1),
                )
        t = persist.tile([H, N], F32, name=f"{nm}sb", tag=nm)
        # sigma-like row: sqrt(scale*sumsq + eps); then reciprocal
        nc.scalar.activation(
            out=t, in_=ps, func=mybir.ActivationFunctionType.Sqrt,
            bias=eps_t, scale=(1.0 if nm == "aq" else 1.0 / D),
        )
        nc.vector.reciprocal(out=t, in_=t)
        stat_sb[nm] = t
    aq_sb, ak_sb = stat_sb["aq"], stat_sb["ak"]
    if use_f32r:
        aq_r = persist.tile([H, N], F32R, name="aq_r", tag="aq_r")
        nc.vector.tensor_copy(out=aq_r, in_=aq_sb)
        aq_sb = aq_r

    # ---- stage C: scale q by broadcast(aq) (folds LN sigma + 1/sqrt(D)) ----
    for r in range(CT):
        ps = psA.tile([128, N], F32, name=f"ps_bq{r}", tag="a")
        for nb in range(NB):
            nc.tensor.matmul(
                ps[:, 512 * nb:512 * (nb + 1)],
                lhsT=mm(ind_sb[:, 128 * r:128 * (r + 1)]),
                rhs=mm(aq_sb[:, 512 * nb:512 * (nb + 1)]),
                start=True, stop=True,
            )
        nc.vector.tensor_mul(out=qh_sb[r], in0=qh_sb[r], in1=ps)

    # ---- stage D: ak columns per m-tile via PE transpose: akT[mt] [128, 12] ----
    akT_sb = []
    for mt in range(NT):
        ps = psB.tile([128, H], F32, name=f"ps_akT{mt}", tag="s")
        nc.tensor.transpose(
            out=ps, in_=ak_sb[:, 128 * mt:128 * (mt + 1)], identity=id12_sb
        )
        t = persist.tile([128, H], F32, name=f"akT{mt}", tag=f"akT{mt}")
        nc.vector.tensor_copy(out=t, in_=ps)
        akT_sb.append(t)

    # ---- stage E: attention, head pairs (row-tiled K=64 matmuls) ----
    ot_sb = []
    for r in range(CT):
        t = xo_pool.tile([128, N], MMD, name=f"ot{r}", tag="xo")
        ot_sb.append(t)

    for pr in range(CT):  # head pair = c-tile of qh/kh
        ot_ps = [
            psA.tile([D + 1, N], F32, name=f"ps_ot{pr}_{j}", tag="a")
            for j in range(2)
        ]
        for mt in range(NT):
            # S^T matmuls: K=64 head pairs at row groups (0,0)/(64,0) run
            # concurrently in the PE array when issued back-to-back.
            s_ps = [
                psB.tile([128, N], F32, name=f"ps_s{2*pr+j}_{mt}", tag="s")
                for j in range(2)
            ]
            for nb in range(NB):
                for j in range(2):
                    nc.tensor.matmul(
                        s_ps[j][:, 512 * nb:512 * (nb + 1)],
                        lhsT=mm(kh_sb[pr][64 * j:64 * (j + 1), 128 * mt:128 * (mt + 1)]),
                        rhs=mm(qh_sb[pr][64 * j:64 * (j + 1), 512 * nb:512 * (nb + 1)]),
                        start=True, stop=True,
                    )
            for j in range(2):
                h = 2 * pr + j
                pt = ptp.tile([128, N], MMD, name=f"pt{h}_{mt}", tag="pt")
                nc.scalar.activation(
                    out=pt, in_=s_ps[j], func=mybir.ActivationFunctionType.Exp,
                    scale=akT_sb[mt][:, h:h + 1],
                )
                for nb in range(NB):
                    nc.tensor.matmul(
                        ot_ps[j][:, 512 * nb:512 * (nb + 1)],
                        lhsT=mm(v_sb[mt][:, h, :]),
                        rhs=mm(pt[:, 512 * nb:512 * (nb + 1)]),
                        start=(mt == 0),
                        stop=(mt == NT - 1),
                    )
        # epilogue: divide by softmax denominator (row 64 of ot_ps)
        for j in range(2):
            h = 2 * pr + j
            tmp = epi.tile([D + 1, N], F32, name=f"tmp{h}", tag="tmp")
            rbc = epi.tile([D, N], F32, name=f"rbc{h}", tag="rbc")
            nc.vector.reciprocal(out=tmp[D:D + 1, :], in_=ot_ps[j][D:D + 1, :])
            dn_dram = dramp.tile([1, N], F32, name=f"dnd{h}", tag="dnd")
            nc.gpsimd.dma_start(out=dn_dram, in_=tmp[D:D + 1, :])
            bcast_ap = bass.AP(
                tensor=dn_dram.tensor, offset=dn_dram.offset,
                ap=[[0, D]] + [list(a) for a in dn_dram.ap[1:]],
            )
            nc.gpsimd.dma_start(out=rbc, in_=bcast_ap)
            if j == 0:
                nc.vector.tensor_mul(
                    out=ot_sb[pr][0:D, :], in0=ot_ps[j][0:D, :], in1=rbc
                )
            else:
                nc.vector.tensor_mul(out=tmp[0:D, :], in0=ot_ps[j][0:D, :], in1=rbc)
                nc.gpsimd.dma_start(out=ot_sb[pr][D:2 * D, :], in_=mm(tmp[0:D, :]))

    # ---- stage F: projection (lands output in natural [n, c] layout) ----
    wp_sb = load_w(wp, "wp", vp_pool, "vp")
    for nt in range(NT):
        ps = psA.tile([128, C], F32, name=f"ps_pj{nt}", tag="a")
        for kc in range(CT):
            for vo, vn in ((0, 512), (512, 256)):
                nc.tensor.matmul(
                    ps[:, vo:vo + vn],
                    lhsT=mm(ot_sb[kc][:, 128 * nt:128 * (nt + 1)]),
                    rhs=mm(wp_sb[kc][:, vo:vo + vn]),
                    start=(kc == 0),
                    stop=(kc == CT - 1),
                )
        osb = work.tile([128, C], F32, name=f"osb{nt}", tag="sq")
        nc.vector.tensor_copy(out=osb, in_=ps)
        nc.sync.dma_start(out=out_ext[128 * nt:128 * (nt + 1), :], in_=osb)


def prep_inputs(x, qkv_weight, proj_weight):
    """CPU-side input preparation: shard, transpose, fold LN centering."""
    x = np.asarray(x, dtype=np.float32)
    qkv_weight = np.asarray(qkv_weight, dtype=np.float32)
    proj_weight = np.asarray(proj_weight, dtype=np.float32)

    Wq = qkv_weight[0:C]
    Wk = qkv_weight[C:2 * C]
    Wv = qkv_weight[2 * C:3 * C]

    def center(W):
        Wc = W.reshape(H, D, C)
        Wc = Wc - Wc.mean(axis=1, keepdims=True)
        return Wc.reshape(C, C)

    wqcT = np.ascontiguousarray(center(Wq).T)
    wkcT = np.ascontiguousarray(center(Wk).T)
    wvT = np.ascontiguousarray(Wv.T)
    wpT = np.ascontiguousarray(proj_weight.T)

    seg = np.zeros((128, CT, H), np.float32)
    for r in range(CT):
        for j in range(2):
            seg[64 * j:64 * (j + 1), r, 2 * r + j] = 1.0
    ind = np.repeat(np.eye(H, dtype=np.float32), D, axis=1)  # [12, 768]
    id12 = np.eye(H, dtype=np.float32)

    in_maps = []
    for b in range(x.shape[0]):
        in_maps.append(dict(
            xT=np.ascontiguousarray(x[b].T),
            wqcT=wqcT, wkcT=wkcT, wvT=wvT, wpT=wpT,
            seg=seg, ind=ind, id12=id12,
        ))
    return in_maps


_CACHE = {}


def kernel(x, qkv_weight, proj_weight):
    if "nc" not in _CACHE:
        _CACHE["nc"] = build()
    nc = _CACHE["nc"]
    in_maps = prep_inputs(x, qkv_weight, proj_weight)
    from concourse.bass_utils import run_bass_kernel_spmd
    res = run_bass_kernel_spmd(nc, in_maps, core_ids=list(range(len(in_maps))))
    out = np.stack([res.results[i]["out"] for i in range(len(in_maps))], axis=0)
    return out.astype(np.float32)


# revision 23
# speedup vs baseline: 1.2160x; 1.2160x over previous
"""Trainium2 Bass kernel for nn_AttentionCompiled (dense transformer attention).

B=8, N=1024, C=768, H=12 heads, D=64. Per-head LayerNorm on q/k, softmax
attention, output projection.

Strategy: pure data parallelism — one batch element per NeuronCore (B=8 ==
n_cores), weights replicated, no collectives.

Math folding (validated vs reference to ~1e-6):
 - LayerNorm centering is folded into the QKV weights: Wq_h <- (I - J/64) Wq_h
   (and same for k). Row-sums of centered vectors vanish, so
   (q-mu_q).(k-mu_k) == qc.kc with both pre-centered by the weights.
 - 1/sigma_q (and the 1/sqrt(D) attention scale) is folded into q before the
   QK matmul: aq[n] = 1/sqrt(sum_d qc^2 + 64*eps) == (1/8)/sqrt(var+eps).
 - 1/sigma_k is applied as the per-partition `scale` operand of the Exp
   activation on S^T tiles (partition axis = m = key index).
 - Softmax has no max-subtraction: |scores| <= 8 by Cauchy-Schwarz on the
   normalized vectors, so exp is always in-range in fp32.
 - Softmax denominators come free from an appended ones-column on V:
   out^T = [v | 1]^T @ P^T gives rows 0..63 = unnormalized output, row 64 =
   sum_m P^T[m, n].
 - All activations live in transposed ([feature, token]) layouts so the
   attention chain needs zero on-chip transposes; x is pre-transposed on the
   CPU (free), and the final projection (lhsT = O^T slice) lands the output
   back in natural [token, feature] layout.
"""

import sys
import numpy as np
from contextlib import ExitStack

if "/opt/trn_rl_repo" not in sys.path:
    sys.path.insert(0, "/opt/trn_rl_repo")

import concourse.bass as bass
import concourse.bacc as bacc
import concourse.tile as tile
from concourse import mybir

F32 = mybir.dt.float32
F32R = mybir.dt.float32r

N = 1024     # tokens per core
C = 768      # channels
H = 12       # heads
D = 64       # head dim
NT = N // 128   # 8 token tiles
CT = C // 128   # 6 channel tiles
NB = N // 512   # 2 free-dim blocks of 512
EPS = 1e-5

USE_F32R = True  # fp32 matmuls run 4 cyc/row; float32r runs 1 cyc/row at N>=256


def build(use_f32r: bool = USE_F32R, compile_module: bool = True,
          repeat: int = 1) -> bass.Bass:
    nc = bacc.Bacc()

    xT = nc.declare_dram_parameter("xT", [C, N], F32, isOutput=False)         # x[b].T
    wq = nc.declare_dram_parameter("wqcT", [C, C], F32, isOutput=False)       # centered Wq^T  [c, hd]
    wk = nc.declare_dram_parameter("wkcT", [C, C], F32, isOutput=False)       # centered Wk^T
    wv = nc.declare_dram_parameter("wvT", [C, C], F32, isOutput=False)        # Wv^T
    wp = nc.declare_dram_parameter("wpT", [C, C], F32, isOutput=False)        # proj_weight^T
    seg = nc.declare_dram_parameter("seg", [128, CT, H], F32, isOutput=False)  # per-c-tile head-segment-sum lhsT
    ind = nc.declare_dram_parameter("ind", [H, C], F32, isOutput=False)       # head->channel block indicator (bcast lhsT)
    id12 = nc.declare_dram_parameter("id12", [H, H], F32, isOutput=False)     # identity for PE transpose
    out_ext = nc.declare_dram_parameter("out", [N, C], F32, isOutput=True)

    MMD = F32R if use_f32r else F32

    def mm(ap):
        # bitcast for APs out of fp32-typed tensors; no-op if already f32r
        if use_f32r and ap.dtype != F32R:
            return ap.bitcast(F32R)
        return ap

    with tile.TileContext(nc) as tc, ExitStack() as ctx:
        # ---- pools ----
        # persistent single-buffer tensors get their own tags in `persist`
        persist = ctx.enter_context(tc.tile_pool(name="persist", bufs=1))
        # xT tiles and O^T tiles share slots (disjoint lifetimes), same for wv/wp
        xo_pool = ctx.enter_context(tc.tile_pool(name="xo", bufs=6))
        vp_pool = ctx.enter_context(tc.tile_pool(name="vp", bufs=6))
        work = ctx.enter_context(tc.tile_pool(name="work", bufs=2))
        ptp = ctx.enter_context(tc.tile_pool(name="ptp", bufs=3))
        epi = ctx.enter_context(tc.tile_pool(name="epi", bufs=2))
        rows = ctx.enter_context(tc.tile_pool(name="rows", bufs=1))
        dramp = ctx.enter_context(tc.tile_pool(name="dramp", bufs=2, space="DRAM"))
        psA = ctx.enter_context(tc.tile_pool(name="psA", bufs=2, space="PSUM"))
        psB = ctx.enter_context(tc.tile_pool(name="psB", bufs=2, space="PSUM"))

        for _it in range(repeat):
            emit_body(nc, tc, use_f32r, mm,
                      xT, wq, wk, wv, wp, seg, ind, id12, out_ext,
                      persist, xo_pool, vp_pool, work, ptp, epi, rows,
                      dramp, psA, psB)

    if compile_module:
        nc.compile()
    return nc


def emit_body(nc, tc, use_f32r, mm,
              xT, wq, wk, wv, wp, seg, ind, id12, out_ext,
              persist, xo_pool, vp_pool, work, ptp, epi, rows,
              dramp, psA, psB):
    MMD = F32R if use_f32r else F32

    # ---- load inputs to SBUF ----
    xt_sb = []
    for r in range(CT):
        t = xo_pool.tile([128, N], MMD, name=f"xt{r}", tag="xo")
        nc.sync.dma_start(out=t, in_=mm(xT[128 * r:128 * (r + 1), :]))
        xt_sb.append(t)

    def load_w(dram, nm, pool, tag, per_tile_tags=False, eng=None):
        tiles = []
        for r in range(CT):
            t = pool.tile(
                [128, C], MMD, name=f"{nm}{r}",
                tag=(f"{tag}{r}" if per_tile_tags else tag),
            )
            (eng or nc.sync).dma_start(out=t, in_=mm(dram[128 * r:128 * (r + 1), :]))
            tiles.append(t)
        return tiles

    # split loads over both HWDGE queues (SP + ACT) to halve the serial
    # prologue: stage A needs xt+wq first, so wq rides the ACT queue while
    # xt streams on SP
    wq_sb = load_w(wq, "wq", persist, "wq", per_tile_tags=True, eng=nc.scalar)
    wk_sb = load_w(wk, "wk", persist, "wk", per_tile_tags=True, eng=nc.scalar)
    wv_sb = load_w(wv, "wv", vp_pool, "vp")

    seg_sb = rows.tile([128, CT, H], MMD, name="seg", tag="seg")
    nc.sync.dma_start(out=seg_sb, in_=mm(seg[:, :, :]))
    ind_sb = rows.tile([H, C], MMD, name="ind", tag="ind")
    nc.sync.dma_start(out=ind_sb, in_=mm(ind[:, :]))
    id12_sb = rows.tile([H, H], F32, name="id12", tag="id12")
    nc.sync.dma_start(out=id12_sb, in_=id12[:, :])

    epsq_sb = rows.tile([H, 1], F32, name="epsq", tag="epsq")
    nc.vector.memset(epsq_sb, float(D) * EPS)
    epsk_sb = rows.tile([H, 1], F32, name="epsk", tag="epsk")
    nc.vector.memset(epsk_sb, EPS)

    # ---- stage A: q^c.T and k^c.T  ([hd, n] layouts, 6 c-tiles each) ----
    qh_sb, kh_sb = [], []
    for nm, wt, outl in (("qh", wq_sb, qh_sb), ("kh", wk_sb, kh_sb)):
        for r in range(CT):
            ps = psA.tile([128, N], F32, name=f"ps_{nm}{r}", tag="a")
            for kc in range(CT):
                for nb in range(NB):
                    nc.tensor.matmul(
                        ps[:, 512 * nb:512 * (nb + 1)],
                        lhsT=mm(wt[kc][:, 128 * r:128 * (r + 1)]),
                        rhs=mm(xt_sb[kc][:, 512 * nb:512 * (nb + 1)]),
                        start=(kc == 0),
                        stop=(kc == CT - 1),
                    )
            t = persist.tile([128, N], MMD, name=f"{nm}sb{r}", tag=f"{nm}{r}")
            nc.vector.tensor_copy(out=t, in_=ps)
            outl.append(t)

    # ---- stage A2: V in natural layout with ones column: [m, h, 65] ----
    v_sb = []
    for mt in range(NT):
        ps = psA.tile([128, C], F32, name=f"ps_v{mt}", tag="a")
        for kc in range(CT):
            for vo, vn in ((0, 512), (512, 256)):
                nc.tensor.matmul(
                    ps[:, vo:vo + vn],
                    lhsT=mm(xt_sb[kc][:, 128 * mt:128 * (mt + 1)]),
                    rhs=mm(wv_sb[kc][:, vo:vo + vn]),
                    start=(kc == 0),
                    stop=(kc == CT - 1),
                )
        t = persist.tile([128, H, D + 1], MMD, name=f"vsb{mt}", tag=f"v{mt}")
        # ones column for the softmax-denominator trick: fill with 1.0
        # first (contiguous memset), then overwrite cols 0..D-1 with v
        nc.vector.memset(t.bitcast(F32) if use_f32r else t, 1.0)
        nc.vector.tensor_copy(
            out=t[:, :, 0:D], in_=ps.rearrange("p (h d) -> p h d", h=H)
        )
        v_sb.append(t)

    # wp rides the ACT queue as soon as the wv slots die (end of stage A2),
    # so the projection weights are resident long before stage F needs them
    wp_sb = load_w(wp, "wp", vp_pool, "vp", eng=nc.scalar)
    # last head's wp rows again at partitions 0:64, so stage F can contract
    # the final pair's odd head straight from its epilogue staging tile
    wp_tail = vp_pool.tile([D, C], MMD, name="wp_tail", tag="wptail", bufs=1)
    nc.scalar.dma_start(out=wp_tail, in_=mm(wp[C - D:C, :]))

    # ---- stage B: per-head inverse std rows  aq, ak [12, 1024] ----
    stat_sb = {}
    for nm, src, eps_t, in (("aq", qh_sb, epsq_sb), ("ak", kh_sb, epsk_sb)):
        ps = psA.tile([H, N], F32, name=f"ps_{nm}", tag="a")
        for r in range(CT):
            sq = work.tile([128, N], MMD, name=f"sq_{nm}{r}", tag="sq")
            nc.vector.tensor_mul(out=sq, in0=src[r], in1=src[r])
            for nb in range(NB):
                nc.tensor.matmul(
                    ps[:, 512 * nb:512 * (nb + 1)],
                    lhsT=mm(seg_sb[:, r, :]),
                    rhs=mm(sq[:, 512 * nb:512 * (nb + 1)]),
                    start=(r == 0),
                    stop=(r == CT - 1),
                )
        t = persist.tile([H, N], F32, name=f"{nm}sb", tag=nm)
        # sigma-like row: sqrt(scale*sumsq + eps); then reciprocal
        nc.scalar.activation(
            out=t, in_=ps, func=mybir.ActivationFunctionType.Sqrt,
            bias=eps_t, scale=(1.0 if nm == "aq" else 1.0 / D),
        )
        nc.vector.reciprocal(out=t, in_=t)
        stat_sb[nm] = t
    aq_sb, ak_sb = stat_sb["aq"], stat_sb["ak"]
    if use_f32r:
        aq_r = persist.tile([H, N], F32R, name="aq_r", tag="aq_r")
        nc.vector.tensor_copy(out=aq_r, in_=aq_sb)
        aq_sb = aq_r

    # ---- stage C: scale q by broadcast(aq) (folds LN sigma + 1/sqrt(D)) ----
    for r in range(CT):
        ps = psA.tile([128, N], F32, name=f"ps_bq{r}", tag="a")
        for nb in range(NB):
            nc.tensor.matmul(
                ps[:, 512 * nb:512 * (nb + 1)],
                lhsT=mm(ind_sb[:, 128 * r:128 * (r + 1)]),
                rhs=mm(aq_sb[:, 512 * nb:512 * (nb + 1)]),
                start=True, stop=True,
            )
        nc.vector.tensor_mul(out=qh_sb[r], in0=qh_sb[r], in1=ps)

    # ---- stage D: ak columns per m-tile via PE transpose: akT[mt] [128, 12] ----
    akT_sb = []
    for mt in range(NT):
        ps = psB.tile([128, H], F32, name=f"ps_akT{mt}", tag="s")
        nc.tensor.transpose(
            out=ps, in_=ak_sb[:, 128 * mt:128 * (mt + 1)], identity=id12_sb
        )
        t = persist.tile([128, H], F32, name=f"akT{mt}", tag=f"akT{mt}")
        nc.vector.tensor_copy(out=t, in_=ps)
        akT_sb.append(t)

    # ---- stage E: attention, head pairs (row-tiled K=64 matmuls) ----
    ot_sb = []
    for r in range(CT):
        t = xo_pool.tile([128, N], MMD, name=f"ot{r}", tag="xo")
        ot_sb.append(t)
    raw_last = None

    for pr in range(CT):  # head pair = c-tile of qh/kh
        ot_ps = [
            psA.tile([D + 1, N], F32, name=f"ps_ot{pr}_{j}", tag="a")
            for j in range(2)
        ]
        for mt in range(NT):
            # S^T matmuls: K=64 head pairs at row groups (0,0)/(64,0) run
            # concurrently in the PE array when issued back-to-back.
            s_ps = [
                psB.tile([128, N], F32, name=f"ps_s{2*pr+j}_{mt}", tag="s")
                for j in range(2)
            ]
            for nb in range(NB):
                for j in range(2):
                    nc.tensor.matmul(
                        s_ps[j][:, 512 * nb:512 * (nb + 1)],
                        lhsT=mm(kh_sb[pr][64 * j:64 * (j + 1), 128 * mt:128 * (mt + 1)]),
                        rhs=mm(qh_sb[pr][64 * j:64 * (j + 1), 512 * nb:512 * (nb + 1)]),
                        start=True, stop=True,
                    )
            for j in range(2):
                h = 2 * pr + j
                pt = ptp.tile([128, N], MMD, name=f"pt{h}_{mt}", tag="pt")
                nc.scalar.activation(
                    out=pt, in_=s_ps[j], func=mybir.ActivationFunctionType.Exp,
                    scale=akT_sb[mt][:, h:h + 1],
                )
                for nb in range(NB):
                    nc.tensor.matmul(
                        ot_ps[j][:, 512 * nb:512 * (nb + 1)],
                        lhsT=mm(v_sb[mt][:, h, :]),
                        rhs=mm(pt[:, 512 * nb:512 * (nb + 1)]),
                        start=(mt == 0),
                        stop=(mt == NT - 1),
                    )
        # epilogue: divide by softmax denominator (row 64 of ot_ps).
        # Stage the raw psum to SBUF first so the psA slots free up
        # immediately for the next head pair's O accumulation.
        for j in range(2):
            h = 2 * pr + j
            raw = epi.tile([D + 1, N], F32, name=f"raw{h}", tag=f"raw{j}", bufs=1)
            # f32r-rounded writes: stage F consumes raw_last as fp32r lhsT
            nc.vector.tensor_copy(out=mm(raw), in_=ot_ps[j])
            # reciprocal lands in rbc row 0, then the idle GPSIMD engine
            # broadcasts it across the 64 partitions — no PSUM slots, no
            # multi-hop DMA latency
            rbc = epi.tile([D, N], F32, name=f"rbc{h}", tag="rbc")
            nc.vector.reciprocal(out=rbc[0:1, :], in_=raw[D:D + 1, :])
            nc.gpsimd.partition_broadcast(rbc, rbc[0:1, :], channels=D)
            if j == 0:
                nc.vector.tensor_mul(
                    out=ot_sb[pr][0:D, :], in0=raw[0:D, :], in1=rbc
                )
            else:
                nc.vector.tensor_mul(out=mm(raw[0:D, :]), in0=raw[0:D, :], in1=rbc)
                if pr < CT - 1:
                    nc.sync.dma_start(
                        out=ot_sb[pr][D:2 * D, :], in_=mm(raw[0:D, :])
                    )
                else:
                    # last pair: skip the partition-shift copy; stage F reads
                    # this head straight out of `raw` as a K=64 lhsT
                    raw_last = raw

    # ---- stage F: projection (lands output in natural [n, c] layout) ----
    for nt in range(NT):
        ps = psA.tile([128, C], F32, name=f"ps_pj{nt}", tag="a")
        for kc in range(CT):
            for vo, vn in ((0, 512), (512, 256)):
                if kc < CT - 1:
                    nc.tensor.matmul(
                        ps[:, vo:vo + vn],
                        lhsT=mm(ot_sb[kc][:, 128 * nt:128 * (nt + 1)]),
                        rhs=mm(wp_sb[kc][:, vo:vo + vn]),
                        start=(kc == 0),
                        stop=False,
                    )
                else:
                    # last c-tile: even head from ot, odd head from the
                    # epilogue staging tile (skips its partition-shift copy)
                    nc.tensor.matmul(
                        ps[:, vo:vo + vn],
                        lhsT=mm(ot_sb[kc][0:D, 128 * nt:128 * (nt + 1)]),
                        rhs=mm(wp_sb[kc][0:D, vo:vo + vn]),
                        start=False, stop=False,
                    )
                    nc.tensor.matmul(
                        ps[:, vo:vo + vn],
                        lhsT=mm(raw_last[0:D, 128 * nt:128 * (nt + 1)]),
                        rhs=mm(wp_tail[:, vo:vo + vn]),
                        start=False, stop=True,
                    )
        osb = work.tile([128, C], F32, name=f"osb{nt}", tag="sq")
        nc.vector.tensor_copy(out=osb, in_=ps)
        out_eng = nc.sync if nt % 2 == 0 else nc.scalar
        out_eng.dma_start(out=out_ext[128 * nt:128 * (nt + 1), :], in_=osb)


def prep_inputs(x, qkv_weight, proj_weight):
    """CPU-side input preparation: shard, transpose, fold LN centering."""
    x = np.asarray(x, dtype=np.float32)
    qkv_weight = np.asarray(qkv_weight, dtype=np.float32)
    proj_weight = np.asarray(proj_weight, dtype=np.float32)

    Wq = qkv_weight[0:C]
    Wk = qkv_weight[C:2 * C]
    Wv = qkv_weight[2 * C:3 * C]

    def center(W):
        Wc = W.reshape(H, D, C)
        Wc = Wc - Wc.mean(axis=1, keepdims=True)
        return Wc.reshape(C, C)

    wqcT = np.ascontiguousarray(center(Wq).T)
    wkcT = np.ascontiguousarray(center(Wk).T)
    wvT = np.ascontiguousarray(Wv.T)
    wpT = np.ascontiguousarray(proj_weight.T)

    seg = np.zeros((128, CT, H), np.float32)
    for r in range(CT):
        for j in range(2):
            seg[64 * j:64 * (j + 1), r, 2 * r + j] = 1.0
    ind = np.repeat(np.eye(H, dtype=np.float32), D, axis=1)  # [12, 768]
    id12 = np.eye(H, dtype=np.float32)

    in_maps = []
    for b in range(x.shape[0]):
        in_maps.append(dict(
            xT=np.ascontiguousarray(x[b].T),
            wqcT=wqcT, wkcT=wkcT, wvT=wvT, wpT=wpT,
            seg=seg, ind=ind, id12=id12,
        ))
    return in_maps


_CACHE = {}


def kernel(x, qkv_weight, proj_weight):
    if "nc" not in _CACHE:
        _CACHE["nc"] = build()
    nc = _CACHE["nc"]
    in_maps = prep_inputs(x, qkv_weight, proj_weight)
    from concourse.bass_utils import run_bass_kernel_spmd
    res = run_bass_kernel_spmd(nc, in_maps, core_ids=list(range(len(in_maps))))
    out = np.stack([res.results[i]["out"] for i in range(len(in_maps))], axis=0)
    return out.astype(np.float32)
